# revision 6
# baseline (speedup 1.0000x reference)
"""Trainium2 Bass kernel for a GPT-style transformer block (B=4, T=1024, C=1024, H=16).

Sharding: 8 cores = (batch b in 0..3) x (sequence half h in 0..1). Each core
computes the full block for its 512 "own" tokens; K/V are computed redundantly
over all 1024 tokens of its batch, so there is no cross-core communication.
Per-core token order is rolled so own tokens are always columns 0:512 — the
SPMD program is identical on every core, only the input data differs.

On-chip layout is channel-major ([C, T], feature dim on partitions) end to end:
every projection contracts over the partition dim, attention computes S^T and
Y^T directly, so no activation transposes are ever needed. LayerNorm gains are
folded into the following weight matrices on the host; LN stats are computed
with fp32r ones-matmuls; matmul operands are bf16 with fp32 PSUM accumulate.
"""

import numpy as np
import ml_dtypes

import concourse.bass as bass
import concourse.bacc as bacc
import concourse.tile as tile
import concourse.mybir as mybir
from concourse.bass_utils import run_bass_kernel_spmd

P = 128
B, T, C, H, D = 4, 1024, 1024, 16, 64
KO = C // P          # 8 contraction chunks of 128 channels
TOWN = T // 2        # 512 own tokens per core
FF = 4 * C

F32 = mybir.dt.float32
F32R = mybir.dt.float32r
BF16 = mybir.dt.bfloat16
np_bf16 = ml_dtypes.bfloat16

Alu = mybir.AluOpType
Act = mybir.ActivationFunctionType

# set by kernel() so an external harness (test.py) can read trace results
TRACE = False
TRACE_KW = {}
LAST_RESULTS = None
_NC_CACHE = None


def _r32(ap):
    return ap.bitcast(F32R)


def _emit(nc, tc, io):
    from contextlib import ExitStack

    with ExitStack() as ctx:
        ep = ctx.enter_context
        consts = ep(tc.tile_pool(name="consts", bufs=1))
        p_wqk = ep(tc.tile_pool(name="p_wqk", bufs=2))
        p_wv = ep(tc.tile_pool(name="p_wv", bufs=9))
        p_wcp = ep(tc.tile_pool(name="p_wcp", bufs=2))
        p_wfc = ep(tc.tile_pool(name="p_wfc", bufs=2))
        p_wpj = ep(tc.tile_pool(name="p_wpj", bufs=3))
        p_big = ep(tc.tile_pool(name="p_big", bufs=2))    # xt_oth / xln / h halves
        p_res = ep(tc.tile_pool(name="p_res", bufs=1))    # xt_own (becomes x2 in place)
        p_act = ep(tc.tile_pool(name="p_act", bufs=1))    # persistent bf16 activations
        p_scr = ep(tc.tile_pool(name="p_scr", bufs=3))    # [P, TOWN] f32 scratch
        p_pt = ep(tc.tile_pool(name="p_pt", bufs=8))      # exp(S^T) chunks, bf16
        p_row = ep(tc.tile_pool(name="p_row", bufs=4))    # [1, TOWN] stat rows
        p_out = ep(tc.tile_pool(name="p_out", bufs=3))    # output staging
        ps_mm = ep(tc.tile_pool(name="ps_mm", bufs=3, space="PSUM"))
        ps_av = ep(tc.tile_pool(name="ps_av", bufs=2, space="PSUM"))
        ps_st = ep(tc.tile_pool(name="ps_st", bufs=3, space="PSUM"))

        # ---- constants / biases ----
        ones_mean = consts.tile([P, 1], F32)        # 1/C  -> ones-matmul = mean
        nc.vector.memset(ones_mean, 1.0 / C)
        ones_row = consts.tile([1, P], F32)         # 1.0  -> partition broadcast matmul
        nc.vector.memset(ones_row, 1.0)
        ones_col_bf = consts.tile([P, 1], BF16)
        nc.vector.memset(ones_col_bf, 1.0)

        bqk_sb = consts.tile([P, 16], F32)
        nc.sync.dma_start(out=bqk_sb, in_=io["bqk"][:])
        bv_sb = consts.tile([P, C], F32)
        nc.sync.dma_start(out=bv_sb, in_=io["bv"][:])
        bcp_sb = consts.tile([P, KO], F32)
        nc.sync.dma_start(out=bcp_sb, in_=io["bcp"][:])
        bfc_sb = consts.tile([P, 32], F32)
        nc.sync.dma_start(out=bfc_sb, in_=io["bfc"][:])
        bpj_sb = consts.tile([P, KO], F32)
        nc.sync.dma_start(out=bpj_sb, in_=io["bpj"][:])

        mask_sb = p_act.tile([P, KO, TOWN], BF16, tag="mask")
        nc.sync.dma_start(out=mask_sb, in_=io["mask"][:])

        # ---- load x^T (channel-major, own half then other half) ----
        xt_own = p_res.tile([P, KO, TOWN], F32, tag="xown")
        xt_oth = p_big.tile([P, KO, TOWN], F32, tag="big")
        for ko in range(KO):
            nc.sync.dma_start(out=xt_own[:, ko, :], in_=io["xt_own"][:, ko, :])
            nc.sync.dma_start(out=xt_oth[:, ko, :], in_=io["xt_oth"][:, ko, :])
        xt = (xt_own, xt_oth)

        # ---- LN1 over all T tokens (stats across partitions via ones-matmuls) ----
        xln = p_big.tile([P, KO, T], BF16, tag="big")

        def emit_ln(src_tiles, dst, dst_col0, ncols):
            """LayerNorm src (tuple of [P,KO,ncols] f32 tiles) -> dst bf16 cols."""
            for s, st in enumerate(src_tiles):
                mu_ps = ps_st.tile([1, TOWN], F32, tag="st")
                for ko in range(KO):
                    nc.tensor.matmul(mu_ps, ones_mean, st[:, ko, :],
                                     start=(ko == 0), stop=(ko == KO - 1))
                mu = p_row.tile([1, TOWN], F32, tag="row")
                nc.scalar.copy(mu, mu_ps)

                sq_ps = ps_st.tile([1, TOWN], F32, tag="st")
                for ko in range(KO):
                    sq = p_scr.tile([P, TOWN], F32, tag="scr")
                    nc.scalar.activation(sq, st[:, ko, :], Act.Square)
                    nc.tensor.matmul(sq_ps, ones_mean, sq,
                                     start=(ko == 0), stop=(ko == KO - 1))
                msq = p_row.tile([1, TOWN], F32, tag="row")
                nc.scalar.copy(msq, sq_ps)

                # rstd = 1 / (sqrt(msq - mu^2) + 1e-5)
                t = p_row.tile([1, TOWN], F32, tag="row")
                nc.vector.tensor_mul(t, mu, mu)
                nc.vector.tensor_sub(t, msq, t)
                nc.scalar.activation(t, t, Act.Sqrt)
                nc.vector.tensor_scalar_add(t, t, 1e-5)
                rstd = p_row.tile([1, TOWN], F32, tag="row")
                nc.vector.reciprocal(rstd, t)

                mu_bc = ps_mm.tile([P, TOWN], F32, tag="mm")
                nc.tensor.matmul(mu_bc, ones_row, mu, start=True, stop=True)
                rs_bc = ps_mm.tile([P, TOWN], F32, tag="mm")
                nc.tensor.matmul(rs_bc, ones_row, rstd, start=True, stop=True)

                c0 = dst_col0 + s * ncols
                for ko in range(KO):
                    tt = p_scr.tile([P, TOWN], F32, tag="scr")
                    nc.vector.tensor_sub(tt, st[:, ko, :], mu_bc)
                    nc.vector.tensor_mul(dst[:, ko, c0:c0 + ncols], tt, rs_bc)

        emit_ln(xt, xln, 0, TOWN)

        # ---- QKV projections (q^T, k^T transposed; v natural) ----
        qT = p_act.tile([P, KO, TOWN], BF16, tag="qT")
        kT = p_act.tile([P, KO, T], BF16, tag="kT")
        for mo in range(16):
            wt = p_wqk.tile([P, KO, P], BF16, tag="wqk")
            nc.sync.dma_start(out=wt, in_=io["wqk"][mo])
            nslices = [(0, TOWN)] if mo < 8 else [(0, TOWN), (TOWN, T)]
            for (a, b) in nslices:
                ps = ps_mm.tile([P, TOWN], F32, tag="mm")
                for ko in range(KO):
                    nc.tensor.matmul(ps, wt[:, ko, :], xln[:, ko, a:b],
                                     start=(ko == 0), stop=(ko == KO - 1))
                if mo < 8:
                    nc.scalar.activation(qT[:, mo, :], ps, Act.Identity,
                                         bias=bqk_sb[:, mo:mo + 1])
                else:
                    nc.scalar.activation(kT[:, mo - 8, a:b], ps, Act.Identity,
                                         bias=bqk_sb[:, mo:mo + 1])

        v_sb = p_act.tile([P, KO, C], BF16, tag="v")
        for nh in range(2):
            wvt = []
            for ko in range(KO):
                w = p_wv.tile([P, TOWN], BF16, tag="wv")
                nc.sync.dma_start(out=w, in_=io["wv"][ko, nh])
                wvt.append(w)
            for tkb in range(KO):
                ps = ps_mm.tile([P, TOWN], F32, tag="mm")
                for ko in range(KO):
                    nc.tensor.matmul(ps, xln[:, ko, tkb * P:(tkb + 1) * P], wvt[ko],
                                     start=(ko == 0), stop=(ko == KO - 1))
                nc.vector.tensor_add(v_sb[:, tkb, nh * TOWN:(nh + 1) * TOWN], ps,
                                     bv_sb[:, nh * TOWN:(nh + 1) * TOWN])

        # ---- attention: S^T = K Q^T per head, exp, mask, Y^T = V^T P^T ----
        yT = p_act.tile([P, KO, TOWN], BF16, tag="yT")
        for hp in range(8):
            pts = {}
            for kc in range(KO):
                for i in range(2):          # head 2hp+i at partitions 64i:64i+64
                    pb = 64 * i
                    ps = ps_mm.tile([P, TOWN], F32, tag="mm")
                    nc.tensor.matmul(ps, kT[pb:pb + 64, hp, kc * P:(kc + 1) * P],
                                     qT[pb:pb + 64, hp, :], start=True, stop=True)
                    pt = p_pt.tile([P, TOWN], BF16, tag="pt")
                    nc.scalar.activation(pt, ps, Act.Exp)
                    nc.vector.tensor_mul(pt, pt, mask_sb[:, kc, :])
                    pts[(i, kc)] = pt
            for i in range(2):
                hd = 2 * hp + i
                pb = 64 * i
                psy = ps_av.tile([P, TOWN], F32, tag="av")
                psz = ps_st.tile([1, TOWN], F32, tag="st")
                for kc in range(KO):
                    nc.tensor.matmul(psy[pb:pb + 64, :],
                                     v_sb[:, kc, hd * 64:(hd + 1) * 64],
                                     pts[(i, kc)],
                                     start=(kc == 0), stop=(kc == KO - 1),
                                     tile_position=(0, pb))
                    nc.tensor.matmul(psz, ones_col_bf, pts[(i, kc)],
                                     start=(kc == 0), stop=(kc == KO - 1))
                z = p_row.tile([1, TOWN], F32, tag="zrow")
                nc.scalar.copy(z, psz)
                rz = p_row.tile([1, TOWN], F32, tag="zrow")
                nc.vector.reciprocal(rz, z)
                rzbc = p_scr.tile([P, TOWN], F32, tag="scr")
                nc.gpsimd.partition_broadcast(rzbc, rz, channels=P)
                nc.vector.tensor_mul(yT[pb:pb + 64, hp, :], psy[pb:pb + 64, :],
                                     rzbc[pb:pb + 64, :])

        # ---- c_proj + residual (x2 written in place over xt_own) ----
        for mo in range(KO):
            wt = p_wcp.tile([P, KO, P], BF16, tag="wcp")
            nc.sync.dma_start(out=wt, in_=io["wcp"][mo])
            ps = ps_mm.tile([P, TOWN], F32, tag="mm")
            for ko in range(KO):
                nc.tensor.matmul(ps, wt[:, ko, :], yT[:, ko, :],
                                 start=(ko == 0), stop=(ko == KO - 1))
            nc.vector.scalar_tensor_tensor(xt_own[:, mo, :], ps,
                                           bcp_sb[:, mo:mo + 1], xt_own[:, mo, :],
                                           op0=Alu.add, op1=Alu.add)

        # ---- LN2 + MLP ----
        x2ln = p_act.tile([P, KO, TOWN], BF16, tag="x2ln")
        emit_ln((xt_own,), x2ln, 0, TOWN)

        h0 = p_big.tile([P, 16, TOWN], BF16, tag="big")
        h1 = p_big.tile([P, 16, TOWN], BF16, tag="big")
        hh = [h0, h1]
        for mo in range(32):
            wt = p_wfc.tile([P, KO, P], BF16, tag="wfc")
            nc.sync.dma_start(out=wt, in_=io["wfc"][mo])
            ps = ps_mm.tile([P, TOWN], F32, tag="mm")
            for ko in range(KO):
                nc.tensor.matmul(ps, wt[:, ko, :], x2ln[:, ko, :],
                                 start=(ko == 0), stop=(ko == KO - 1))
            nc.scalar.activation(hh[mo // 16][:, mo % 16, :], ps, Act.Gelu,
                                 bias=bfc_sb[:, mo:mo + 1])

        for mo in range(KO):
            wts = []
            for half in range(2):
                wt = p_wpj.tile([P, 16, P], BF16, tag="wpj")
                nc.sync.dma_start(out=wt, in_=io["wpj"][mo][:, half * 16:(half + 1) * 16, :])
                wts.append(wt)
            ps = ps_mm.tile([P, TOWN], F32, tag="mm")
            for ko in range(32):
                nc.tensor.matmul(ps, wts[ko // 16][:, ko % 16, :],
                                 hh[ko // 16][:, ko % 16, :],
                                 start=(ko == 0), stop=(ko == 31))
            ot = p_out.tile([P, TOWN], F32, tag="outst")
            nc.vector.scalar_tensor_tensor(ot, ps, bpj_sb[:, mo:mo + 1],
                                           xt_own[:, mo, :],
                                           op0=Alu.add, op1=Alu.add)
            nc.sync.dma_start(out=io["out"][:, mo, :], in_=ot)


def _build_nc():
    nc = bacc.Bacc("TRN2", target_bir_lowering=False, debug=False)
    io = {}
    dt = nc.dram_tensor
    io["xt_own"] = dt("xt_own", [P, KO, TOWN], F32, kind="ExternalInput")
    io["xt_oth"] = dt("xt_oth", [P, KO, TOWN], F32, kind="ExternalInput")
    io["wqk"] = dt("wqk", [16, P, KO, P], BF16, kind="ExternalInput")
    io["wv"] = dt("wv", [KO, 2, P, TOWN], BF16, kind="ExternalInput")
    io["wcp"] = dt("wcp", [KO, P, KO, P], BF16, kind="ExternalInput")
    io["wfc"] = dt("wfc", [32, P, KO, P], BF16, kind="ExternalInput")
    io["wpj"] = dt("wpj", [KO, P, 32, P], BF16, kind="ExternalInput")
    io["bqk"] = dt("bqk", [P, 16], F32, kind="ExternalInput")
    io["bv"] = dt("bv", [P, C], F32, kind="ExternalInput")
    io["bcp"] = dt("bcp", [P, KO], F32, kind="ExternalInput")
    io["bfc"] = dt("bfc", [P, 32], F32, kind="ExternalInput")
    io["bpj"] = dt("bpj", [P, KO], F32, kind="ExternalInput")
    io["mask"] = dt("mask", [P, KO, TOWN], BF16, kind="ExternalInput")
    io["out"] = dt("out", [P, KO, TOWN], F32, kind="ExternalOutput")
    with tile.TileContext(nc) as tc:
        _emit(nc, tc, io)
    nc.compile()
    return nc


def _prep_maps(inputs):
    f32 = np.float32
    g = {k: np.asarray(v, f32) for k, v in inputs.items()}

    # fold LN gains/biases into the following projections
    Wa = g["c_attn_w"] * g["ln1_w"][:, None]
    ba = g["c_attn_b"] + g["ln1_b"] @ g["c_attn_w"]
    Wq, Wk, Wv = Wa[:, :C] * 0.125, Wa[:, C:2 * C], Wa[:, 2 * C:]
    bq, bk, bv = ba[:C] * 0.125, ba[C:2 * C], ba[2 * C:]
    Wfc = g["fc_w"] * g["ln2_w"][:, None]
    bfc = g["fc_b"] + g["ln2_b"] @ g["fc_w"]

    def lhsT_arrange(w, n_mo):  # [C_in, N] -> [n_mo, P(ki), KO_in, P(mi)] bf16
        ko_in = w.shape[0] // P
        return np.ascontiguousarray(
            w.reshape(ko_in, P, n_mo, P).transpose(2, 1, 0, 3)).astype(np_bf16)

    shared = {
        "wqk": lhsT_arrange(np.concatenate([Wq, Wk], axis=1), 16),
        "wv": np.ascontiguousarray(
            Wv.reshape(KO, P, 2, TOWN).transpose(0, 2, 1, 3)).astype(np_bf16),
        "wcp": lhsT_arrange(g["c_proj_w"], KO),
        "wfc": lhsT_arrange(Wfc, 32),
        "wpj": lhsT_arrange(g["proj_w"], KO),
        "bqk": np.ascontiguousarray(
            np.concatenate([bq, bk]).reshape(16, P).T).astype(f32),
        "bv": np.ascontiguousarray(np.broadcast_to(bv, (P, C))).astype(f32),
        "bcp": np.ascontiguousarray(g["c_proj_b"].reshape(KO, P).T).astype(f32),
        "bfc": np.ascontiguousarray(bfc.reshape(32, P).T).astype(f32),
        "bpj": np.ascontiguousarray(g["proj_b"].reshape(KO, P).T).astype(f32),
    }

    maps = []
    gq_base = np.arange(TOWN)
    gk_base = np.arange(T)
    for c in range(8):
        b, h = divmod(c, 2)
        xr = np.roll(g["x"][b], -h * TOWN, axis=0)          # own tokens first
        arr = np.ascontiguousarray(
            xr.T.reshape(KO, P, T).transpose(1, 0, 2)).astype(f32)  # [P, KO, T]
        gk = (gk_base + h * TOWN) % T
        gq = h * TOWN + gq_base
        m = (gk[:, None] <= gq[None, :]).astype(f32)         # [T, TOWN]
        mask = np.ascontiguousarray(
            m.reshape(KO, P, TOWN).transpose(1, 0, 2)).astype(np_bf16)
        maps.append(dict(shared,
                         xt_own=np.ascontiguousarray(arr[:, :, :TOWN]),
                         xt_oth=np.ascontiguousarray(arr[:, :, TOWN:]),
                         mask=mask))
    return maps


def kernel(**inputs):
    global LAST_RESULTS, _NC_CACHE
    if _NC_CACHE is None:
        _NC_CACHE = _build_nc()
    nc = _NC_CACHE
    maps = _prep_maps(inputs)
    res = run_bass_kernel_spmd(nc, maps, core_ids=list(range(8)),
                               trace=TRACE, **TRACE_KW)
    LAST_RESULTS = res
    out = np.zeros((B, T, C), np.float32)
    for c in range(8):
        b, h = divmod(c, 2)
        ot = res.results[c]["out"]                # [P, KO, TOWN]
        out[b, h * TOWN:(h + 1) * TOWN, :] = \
            ot.transpose(1, 0, 2).reshape(C, TOWN).T
    return out


# revision 19
# speedup vs baseline: 1.3150x; 1.3150x over previous
"""Trainium2 Bass kernel for a GPT-style transformer block (B=4, T=1024, C=1024, H=16).

Sharding: 8 cores = (batch b in 0..3) x (sequence half h in 0..1). Each core
computes the full block for its 512 "own" tokens; K/V are computed redundantly
over all 1024 tokens of its batch, so there is no cross-core communication.
Per-core token order is rolled so own tokens are always columns 0:512 — the
SPMD program is identical on every core, only the input data differs.

On-chip layout is channel-major ([C, T], feature dim on partitions) end to end:
every projection contracts over the partition dim, attention computes S^T and
Y^T directly, so no activation transposes are ever needed. LayerNorm gains are
folded into the following weight matrices on the host; LN stats are computed
with bf16 ones-matmuls (fp32 PSUM accumulate); matmul operands are bf16 with fp32 PSUM accumulate.
"""

import numpy as np
import ml_dtypes

import concourse.bass as bass
import concourse.bacc as bacc
import concourse.tile as tile
import concourse.mybir as mybir
from concourse.bass_utils import run_bass_kernel_spmd

P = 128
B, T, C, H, D = 4, 1024, 1024, 16, 64
KO = C // P          # 8 contraction chunks of 128 channels
TOWN = T // 2        # 512 own tokens per core
FF = 4 * C

F32 = mybir.dt.float32
F32R = mybir.dt.float32r
BF16 = mybir.dt.bfloat16
np_bf16 = ml_dtypes.bfloat16

Alu = mybir.AluOpType
Act = mybir.ActivationFunctionType

# set by kernel() so an external harness (test.py) can read trace results
TRACE = False
TRACE_KW = {}
LAST_RESULTS = None
_NC_CACHE = None


def _r32(ap):
    return ap.bitcast(F32R)


def _emit(nc, tc, io):
    from contextlib import ExitStack

    T2 = 2 * TOWN
    with ExitStack() as ctx:
        ep = ctx.enter_context
        consts = ep(tc.tile_pool(name="consts", bufs=1))
        p_wqk = ep(tc.tile_pool(name="p_wqk", bufs=2))
        p_wv = ep(tc.tile_pool(name="p_wv", bufs=9))
        p_wcp = ep(tc.tile_pool(name="p_wcp", bufs=2))
        p_wfc = ep(tc.tile_pool(name="p_wfc", bufs=2))
        p_wpj = ep(tc.tile_pool(name="p_wpj", bufs=3))
        p_big = ep(tc.tile_pool(name="p_big", bufs=2))    # xt_oth / xln / h halves
        p_res = ep(tc.tile_pool(name="p_res", bufs=1))    # xt_own (becomes x2 in place)
        p_act = ep(tc.tile_pool(name="p_act", bufs=1))    # persistent bf16 activations
        p_scr = ep(tc.tile_pool(name="p_scr", bufs=3))    # [P, TOWN] f32 scratch
        p_pt = ep(tc.tile_pool(name="p_pt", bufs=12))     # exp(S^T) kc-pair chunks
        p_row = ep(tc.tile_pool(name="p_row", bufs=3))    # [1, TOWN] stat rows
        p_out = ep(tc.tile_pool(name="p_out", bufs=2))    # output staging
        ps_mm = ep(tc.tile_pool(name="ps_mm", bufs=3, space="PSUM"))   # [P,1024] = 2 banks
        ps_av = ep(tc.tile_pool(name="ps_av", bufs=2, space="PSUM"))   # [P,512] = 1 bank

        # ---- constants / biases ----
        ones_mean_bf = consts.tile([P, 1], BF16)    # 1/C  -> ones-matmul = mean
        nc.vector.memset(ones_mean_bf, 1.0 / C)
        ones_row = consts.tile([1, P], F32)         # 1.0  -> partition broadcast matmul
        nc.vector.memset(ones_row, 1.0)

        bqk_sb = consts.tile([P, 16], F32)
        nc.sync.dma_start(out=bqk_sb, in_=io["bqk"][:])
        bv_sb = consts.tile([P, C], F32)
        nc.sync.dma_start(out=bv_sb, in_=io["bv"][:])
        bcp_sb = consts.tile([P, KO], F32)
        nc.sync.dma_start(out=bcp_sb, in_=io["bcp"][:])
        bfc_sb = consts.tile([P, 32], F32)
        nc.sync.dma_start(out=bfc_sb, in_=io["bfc"][:])
        bpj_sb = consts.tile([P, KO], F32)
        nc.sync.dma_start(out=bpj_sb, in_=io["bpj"][:])

        mask_sb = p_act.tile([P, 2, T2], BF16, tag="mask")   # kc-pair packed tril
        nc.sync.dma_start(out=mask_sb, in_=io["mask"][:])
        ebias_sb = consts.tile([P, 1], F32)
        nc.sync.dma_start(out=ebias_sb, in_=io["ebias"][:])

        # ---- load x^T: bf16 full (LN/QKV path) + f32 own half (residual) ----
        xt_own = p_res.tile([P, KO, TOWN], F32, tag="xown")
        x_bf = p_big.tile([P, KO, T], BF16, tag="big")
        for ko in range(KO):
            nc.gpsimd.dma_start(out=x_bf[:, ko, :], in_=io["x_bf"][:, ko, :])
            nc.sync.dma_start(out=xt_own[:, ko, :], in_=io["xt_own"][:, ko, :])

        # ---- LayerNorm (stats across partitions via bf16 ones-matmuls) ----
        xln = p_big.tile([P, KO, T], BF16, tag="big")

        def emit_ln(srcs, dst, src_is_bf, stats_ps=None):
            """srcs: list of (tile, col0); normalizes [P,KO,TOWN] col-slices."""
            for s, (st, sc0) in enumerate(srcs):
                if stats_ps is not None:
                    st_ps = stats_ps
                else:
                    st_ps = ps_mm.tile([P, T2], F32, tag="mm")
                mu_ps = st_ps[0:1, 0:TOWN]
                sq_ps = st_ps[0:1, TOWN:T2]
                for ko in range(0 if stats_ps is not None else KO):
                    if src_is_bf:
                        xb = st[:, ko, sc0:sc0 + TOWN]
                    else:
                        xb = p_scr.tile([P, TOWN], BF16, tag="scr")
                        nc.scalar.copy(xb, st[:, ko, sc0:sc0 + TOWN])
                    sq = p_scr.tile([P, TOWN], BF16, tag="scr")
                    nc.vector.tensor_mul(sq, xb, xb)
                    nc.tensor.matmul(mu_ps, ones_mean_bf, xb,
                                     start=(ko == 0), stop=(ko == KO - 1))
                    nc.tensor.matmul(sq_ps, ones_mean_bf, sq,
                                     start=(ko == 0), stop=(ko == KO - 1))
                mu = p_row.tile([1, TOWN], F32, tag="row")
                nc.scalar.copy(mu, mu_ps)
                msq = p_row.tile([1, TOWN], F32, tag="row")
                nc.scalar.copy(msq, sq_ps)

                # rstd = 1 / (sqrt(msq - mu^2) + 1e-5)
                t = p_row.tile([1, TOWN], F32, tag="row")
                nc.vector.tensor_mul(t, mu, mu)
                nc.vector.tensor_sub(t, msq, t)
                nc.scalar.activation(t, t, Act.Sqrt)
                nc.vector.tensor_scalar_add(t, t, 1e-5)
                rstd = p_row.tile([1, TOWN], F32, tag="row")
                nc.vector.reciprocal_approx_fast(rstd, t)

                bc_ps = ps_mm.tile([P, T2], F32, tag="mm")
                mu_bc = bc_ps[:, 0:TOWN]
                rs_bc = bc_ps[:, TOWN:T2]
                nc.tensor.matmul(mu_bc, ones_row, mu, start=True, stop=True)
                nc.tensor.matmul(rs_bc, ones_row, rstd, start=True, stop=True)

                for ko in range(KO):
                    tt = p_scr.tile([P, TOWN], F32, tag="scr")
                    nc.vector.tensor_sub(tt, st[:, ko, sc0:sc0 + TOWN], mu_bc)
                    nc.vector.tensor_mul(dst[:, ko, sc0:sc0 + TOWN], tt, rs_bc)

        emit_ln([(x_bf, 0), (x_bf, TOWN)], xln, True)

        # ---- QKV projections (q^T, k^T transposed; v natural) ----
        qT = p_act.tile([P, KO, TOWN], BF16, tag="qT")
        kT = p_act.tile([P, KO, T], BF16, tag="kT")
        # q: pairs of output-channel chunks share one 2-bank psum tile
        for mop in range(4):
            ps = ps_mm.tile([P, T2], F32, tag="mm")
            for half in range(2):
                mo = 2 * mop + half
                wt = p_wqk.tile([P, KO, P], BF16, tag="wqk")
                (nc.sync if mo % 2 == 0 else nc.gpsimd).dma_start(
                    out=wt, in_=io["wqk"][mo])
                for ko in range(KO):
                    nc.tensor.matmul(ps[:, half * TOWN:(half + 1) * TOWN],
                                     wt[:, ko, :], xln[:, ko, 0:TOWN],
                                     start=(ko == 0), stop=(ko == KO - 1))
            for half in range(2):
                mo = 2 * mop + half
                nc.scalar.activation(qT[:, mo, :],
                                     ps[:, half * TOWN:(half + 1) * TOWN],
                                     Act.Identity, bias=bqk_sb[:, mo:mo + 1])
        # k: one chunk's own+oth halves share a tile; single batched evict
        for mo in range(8, 16):
            wt = p_wqk.tile([P, KO, P], BF16, tag="wqk")
            (nc.sync if mo % 2 == 0 else nc.gpsimd).dma_start(
                out=wt, in_=io["wqk"][mo])
            ps = ps_mm.tile([P, T2], F32, tag="mm")
            for half in range(2):
                for ko in range(KO):
                    nc.tensor.matmul(ps[:, half * TOWN:(half + 1) * TOWN],
                                     wt[:, ko, :],
                                     xln[:, ko, half * TOWN:(half + 1) * TOWN],
                                     start=(ko == 0), stop=(ko == KO - 1))
            nc.scalar.activation(kT[:, mo - 8, :], ps, Act.Identity,
                                 bias=bqk_sb[:, mo:mo + 1])

        v_ext = p_act.tile([P, KO, 16 * 65], BF16, tag="v")
        nc.vector.memset(v_ext, 1.0)
        for nh in range(2):
            wvt = []
            for ko in range(KO):
                w = p_wv.tile([P, TOWN], BF16, tag="wv")
                (nc.sync if ko % 2 == 0 else nc.gpsimd).dma_start(
                    out=w, in_=io["wv"][ko, nh])
                wvt.append(w)
            for tkbp in range(4):
                ps = ps_mm.tile([P, T2], F32, tag="mm")
                for half in range(2):
                    tkb = 2 * tkbp + half
                    for ko in range(KO):
                        nc.tensor.matmul(ps[:, half * TOWN:(half + 1) * TOWN],
                                         xln[:, ko, tkb * P:(tkb + 1) * P],
                                         wvt[ko],
                                         start=(ko == 0), stop=(ko == KO - 1))
                for half in range(2):
                    tkb = 2 * tkbp + half
                    vout = v_ext[:, tkb].rearrange("p (h d) -> p h d", d=65)
                    nc.vector.tensor_add(
                        vout[:, nh * 8:(nh + 1) * 8, 0:64],
                        ps[:, half * TOWN:(half + 1) * TOWN].rearrange(
                            "p (h d) -> p h d", d=64),
                        bv_sb[:, nh * TOWN:(nh + 1) * TOWN].rearrange(
                            "p (h d) -> p h d", d=64))

        # ---- attention ----
        yT = p_act.tile([P, KO, TOWN], BF16, tag="yT")
        all_pts = {}

        def emit_scores(hp):
            for i in range(2):              # head 2hp+i at partitions 64i:64i+64
                pb = 64 * i
                for kcp in range(4):        # kc pair (2kcp, 2kcp+1)
                    ps = ps_mm.tile([P, T2], F32, tag="mm")
                    for half in range(2):
                        kc = 2 * kcp + half
                        nc.tensor.matmul(ps[:, half * TOWN:(half + 1) * TOWN],
                                         kT[pb:pb + 64, hp, kc * P:(kc + 1) * P],
                                         qT[pb:pb + 64, hp, :],
                                         start=True, stop=True)
                    pt = p_pt.tile([P, T2], BF16, tag="pt")
                    if kcp < 2:
                        nc.scalar.activation(pt, ps, Act.Exp)
                        nc.vector.tensor_mul(pt, pt, mask_sb[:, kcp, :])
                    else:
                        nc.scalar.activation(pt, ps, Act.Exp,
                                             bias=ebias_sb[:, 0:1])
                    all_pts[(hp, i, kcp)] = pt

        def emit_av(hp):
            psy_a = ps_av.tile([P, TOWN], F32, tag="av")
            psy_b = ps_av.tile([P, TOWN], F32, tag="av")
            psy = [psy_a, psy_b]
            for i in range(2):
                hd = 2 * hp + i
                for kc in range(KO):
                    pt = all_pts[(hp, i, kc // 2)]
                    nc.tensor.matmul(psy[i][0:65, :],
                                     v_ext[:, kc, hd * 65:(hd + 1) * 65],
                                     pt[:, (kc % 2) * TOWN:(kc % 2 + 1) * TOWN],
                                     start=(kc == 0), stop=(kc == KO - 1))
            for i in range(2):
                pb = 64 * i
                z = p_row.tile([1, TOWN], F32, tag="zrow")
                nc.vector.tensor_copy(z, psy[i][64:65, :])
                rz = p_row.tile([1, TOWN], F32, tag="zrow")
                nc.vector.reciprocal_approx_fast(rz, z)
                rzbc = p_scr.tile([P, TOWN], F32, tag="scr")
                nc.gpsimd.partition_broadcast(rzbc, rz, channels=P)
                nc.vector.tensor_mul(yT[pb:pb + 64, hp, :], psy[i][0:64, :],
                                     rzbc[0:64, :])

        emit_scores(0)
        for hp in range(1, 8):
            emit_scores(hp)
            emit_av(hp - 1)
        emit_av(7)

        # ---- c_proj + residual (x2 written in place over xt_own) ----
        for mop in range(4):
            ps = ps_mm.tile([P, T2], F32, tag="mm")
            for half in range(2):
                mo = 2 * mop + half
                wt = p_wcp.tile([P, KO, P], BF16, tag="wcp")
                nc.sync.dma_start(out=wt, in_=io["wcp"][mo])
                for ko in range(KO):
                    nc.tensor.matmul(ps[:, half * TOWN:(half + 1) * TOWN],
                                     wt[:, ko, :], yT[:, ko, :],
                                     start=(ko == 0), stop=(ko == KO - 1))
            for half in range(2):
                mo = 2 * mop + half
                nc.vector.scalar_tensor_tensor(
                    xt_own[:, mo, :], ps[:, half * TOWN:(half + 1) * TOWN],
                    bcp_sb[:, mo:mo + 1], xt_own[:, mo, :],
                    op0=Alu.add, op1=Alu.add)

        # ---- LN2 + MLP ----
        x2ln = p_act.tile([P, KO, TOWN], BF16, tag="x2ln")
        emit_ln([(xt_own, 0)], x2ln, False)

        h0 = p_big.tile([P, 16, TOWN], BF16, tag="big")
        h1 = p_big.tile([P, 16, TOWN], BF16, tag="big")
        hh = [h0, h1]
        for mop in range(16):
            ps = ps_mm.tile([P, T2], F32, tag="mm")
            for half in range(2):
                mo = 2 * mop + half
                wt = p_wfc.tile([P, KO, P], BF16, tag="wfc")
                (nc.sync if mo % 2 == 0 else nc.gpsimd).dma_start(
                    out=wt, in_=io["wfc"][mo])
                for ko in range(KO):
                    nc.tensor.matmul(ps[:, half * TOWN:(half + 1) * TOWN],
                                     wt[:, ko, :], x2ln[:, ko, :],
                                     start=(ko == 0), stop=(ko == KO - 1))
            for half in range(2):
                mo = 2 * mop + half
                nc.scalar.activation(hh[mo // 16][:, mo % 16, :],
                                     ps[:, half * TOWN:(half + 1) * TOWN],
                                     Act.Gelu, bias=bfc_sb[:, mo:mo + 1])

        for mop in range(4):
            ps = ps_mm.tile([P, T2], F32, tag="mm")
            for half in range(2):
                mo = 2 * mop + half
                wts = []
                for whalf in range(2):
                    wt = p_wpj.tile([P, 16, P], BF16, tag="wpj")
                    (nc.sync if whalf == 0 else nc.gpsimd).dma_start(
                        out=wt, in_=io["wpj"][mo][:, whalf * 16:(whalf + 1) * 16, :])
                    wts.append(wt)
                for ko in range(32):
                    nc.tensor.matmul(ps[:, half * TOWN:(half + 1) * TOWN],
                                     wts[ko // 16][:, ko % 16, :],
                                     hh[ko // 16][:, ko % 16, :],
                                     start=(ko == 0), stop=(ko == 31))
            for half in range(2):
                mo = 2 * mop + half
                ot = p_out.tile([P, TOWN], F32, tag="outst")
                nc.vector.scalar_tensor_tensor(ot, ps[:, half * TOWN:(half + 1) * TOWN],
                                               bpj_sb[:, mo:mo + 1],
                                               xt_own[:, mo, :],
                                               op0=Alu.add, op1=Alu.add)
                nc.sync.dma_start(out=io["out"][:, mo, :], in_=ot)


def _build_nc():
    nc = bacc.Bacc("TRN2", target_bir_lowering=False, debug=False)
    io = {}
    dt = nc.dram_tensor
    io["xt_own"] = dt("xt_own", [P, KO, TOWN], F32, kind="ExternalInput")
    io["x_bf"] = dt("x_bf", [P, KO, T], BF16, kind="ExternalInput")
    io["wqk"] = dt("wqk", [16, P, KO, P], BF16, kind="ExternalInput")
    io["wv"] = dt("wv", [KO, 2, P, TOWN], BF16, kind="ExternalInput")
    io["wcp"] = dt("wcp", [KO, P, KO, P], BF16, kind="ExternalInput")
    io["wfc"] = dt("wfc", [32, P, KO, P], BF16, kind="ExternalInput")
    io["wpj"] = dt("wpj", [KO, P, 32, P], BF16, kind="ExternalInput")
    io["bqk"] = dt("bqk", [P, 16], F32, kind="ExternalInput")
    io["bv"] = dt("bv", [P, C], F32, kind="ExternalInput")
    io["bcp"] = dt("bcp", [P, KO], F32, kind="ExternalInput")
    io["bfc"] = dt("bfc", [P, 32], F32, kind="ExternalInput")
    io["bpj"] = dt("bpj", [P, KO], F32, kind="ExternalInput")
    io["mask"] = dt("mask", [P, 2, T], BF16, kind="ExternalInput")
    io["ebias"] = dt("ebias", [P, 1], F32, kind="ExternalInput")
    io["out"] = dt("out", [P, KO, TOWN], F32, kind="ExternalOutput")
    with tile.TileContext(nc) as tc:
        _emit(nc, tc, io)
    nc.compile()
    return nc


def _prep_maps(inputs):
    f32 = np.float32
    g = {k: np.asarray(v, f32) for k, v in inputs.items()}

    # fold LN gains/biases into the following projections
    Wa = g["c_attn_w"] * g["ln1_w"][:, None]
    ba = g["c_attn_b"] + g["ln1_b"] @ g["c_attn_w"]
    Wq, Wk, Wv = Wa[:, :C] * 0.125, Wa[:, C:2 * C], Wa[:, 2 * C:]
    bq, bk, bv = ba[:C] * 0.125, ba[C:2 * C], ba[2 * C:]
    Wfc = g["fc_w"] * g["ln2_w"][:, None]
    bfc = g["fc_b"] + g["ln2_b"] @ g["fc_w"]

    def lhsT_arrange(w, n_mo):  # [C_in, N] -> [n_mo, P(ki), KO_in, P(mi)] bf16
        ko_in = w.shape[0] // P
        return np.ascontiguousarray(
            w.reshape(ko_in, P, n_mo, P).transpose(2, 1, 0, 3)).astype(np_bf16)

    shared = {
        "wqk": lhsT_arrange(np.concatenate([Wq, Wk], axis=1), 16),
        "wv": np.ascontiguousarray(
            Wv.reshape(KO, P, 2, TOWN).transpose(0, 2, 1, 3)).astype(np_bf16),
        "wcp": lhsT_arrange(g["c_proj_w"], KO),
        "wfc": lhsT_arrange(Wfc, 32),
        "wpj": lhsT_arrange(g["proj_w"], KO),
        "bqk": np.ascontiguousarray(
            np.concatenate([bq, bk]).reshape(16, P).T).astype(f32),
        "bv": np.ascontiguousarray(np.broadcast_to(bv, (P, C))).astype(f32),
        "bcp": np.ascontiguousarray(g["c_proj_b"].reshape(KO, P).T).astype(f32),
        "bfc": np.ascontiguousarray(bfc.reshape(32, P).T).astype(f32),
        "bpj": np.ascontiguousarray(g["proj_b"].reshape(KO, P).T).astype(f32),
    }

    maps = []
    gq_base = np.arange(TOWN)
    gk_base = np.arange(T)
    for c in range(8):
        b, h = divmod(c, 2)
        xr = np.roll(g["x"][b], -h * TOWN, axis=0)          # own tokens first
        arr = np.ascontiguousarray(
            xr.T.reshape(KO, P, T).transpose(1, 0, 2)).astype(f32)  # [P, KO, T]
        m = (gk_base[:TOWN, None] <= gq_base[None, :]).astype(f32)  # tril [TOWN, TOWN]
        # [P(ki), kcp, half*TOWN+q] with key = (2*kcp+half)*P + ki
        mask = np.ascontiguousarray(
            m.reshape(2, 2, P, TOWN).transpose(2, 0, 1, 3).reshape(P, 2, T)
        ).astype(np_bf16)
        ebias = np.full((P, 1), -50.0 if h == 0 else 0.0, f32)
        maps.append(dict(shared,
                         xt_own=np.ascontiguousarray(arr[:, :, :TOWN]),
                         x_bf=arr.astype(np_bf16),
                         mask=mask, ebias=ebias))
    return maps


def kernel(**inputs):
    global LAST_RESULTS, _NC_CACHE
    if _NC_CACHE is None:
        _NC_CACHE = _build_nc()
    nc = _NC_CACHE
    maps = _prep_maps(inputs)
    res = run_bass_kernel_spmd(nc, maps, core_ids=list(range(8)),
                               trace=TRACE, **TRACE_KW)
    LAST_RESULTS = res
    out = np.zeros((B, T, C), np.float32)
    for c in range(8):
        b, h = divmod(c, 2)
        ot = res.results[c]["out"]                # [P, KO, TOWN]
        out[b, h * TOWN:(h + 1) * TOWN, :] = \
            ot.transpose(1, 0, 2).reshape(C, TOWN).T
    return out


# revision 25
# speedup vs baseline: 1.5535x; 1.1814x over previous
"""Trainium2 Bass kernel for a GPT-style transformer block (B=4, T=1024, C=1024, H=16).

Sharding: 8 cores = (batch b in 0..3) x (sequence half h in 0..1). Each core
computes the full block for its 512 "own" tokens; K/V are computed redundantly
over all 1024 tokens of its batch, so there is no cross-core communication.
Per-core token order is rolled so own tokens are always columns 0:512 — the
SPMD program is identical on every core, only the input data differs.

On-chip layout is channel-major ([C, T], feature dim on partitions) end to end:
every projection contracts over the partition dim, attention computes S^T and
Y^T directly, so no activation transposes are ever needed. LayerNorm gains are
folded into the following weight matrices on the host; LN stats use bf16 ones-matmuls; all matmul operands are bf16 with
fp32 PSUM accumulation.
"""

import numpy as np
import ml_dtypes

import concourse.bass as bass
import concourse.bacc as bacc
import concourse.tile as tile
import concourse.mybir as mybir
from concourse.bass_utils import run_bass_kernel_spmd

P = 128
B, T, C, H, D = 4, 1024, 1024, 16, 64
KO = C // P          # 8 contraction chunks of 128 channels
TOWN = T // 2        # 512 own tokens per core
FF = 4 * C

F32 = mybir.dt.float32
F32R = mybir.dt.float32r
BF16 = mybir.dt.bfloat16
np_bf16 = ml_dtypes.bfloat16

Alu = mybir.AluOpType
Act = mybir.ActivationFunctionType

# set by kernel() so an external harness (test.py) can read trace results
TRACE = False
TRACE_KW = {}
LAST_RESULTS = None
_NC_CACHE = None


def _r32(ap):
    return ap.bitcast(F32R)


def _emit(nc, tc, io):
    from contextlib import ExitStack

    T2 = 2 * TOWN
    with ExitStack() as ctx:
        ep = ctx.enter_context
        consts = ep(tc.tile_pool(name="consts", bufs=1))
        p_wqk = ep(tc.tile_pool(name="p_wqk", bufs=4))
        p_wv = ep(tc.tile_pool(name="p_wv", bufs=9))
        p_wcp = ep(tc.tile_pool(name="p_wcp", bufs=4))
        p_wfc = ep(tc.tile_pool(name="p_wfc", bufs=3))
        p_wpj = ep(tc.tile_pool(name="p_wpj", bufs=4))
        p_big = ep(tc.tile_pool(name="p_big", bufs=2))    # xt_oth / xln / h halves
        p_res = ep(tc.tile_pool(name="p_res", bufs=1))    # xt_own (becomes x2 in place)
        p_act = ep(tc.tile_pool(name="p_act", bufs=1))    # persistent bf16 activations
        p_scr = ep(tc.tile_pool(name="p_scr", bufs=3))    # [P, TOWN] f32 scratch
        p_pt = ep(tc.tile_pool(name="p_pt", bufs=12))     # exp(S^T) kc-pair chunks
        p_row = ep(tc.tile_pool(name="p_row", bufs=3))    # [1, TOWN] stat rows
        p_out = ep(tc.tile_pool(name="p_out", bufs=2))    # output staging
        ps_mm = ep(tc.tile_pool(name="ps_mm", bufs=3, space="PSUM"))   # [P,1024] = 2 banks
        ps_av = ep(tc.tile_pool(name="ps_av", bufs=2, space="PSUM"))   # [P,512] = 1 bank

        # ---- constants / biases ----
        ones_mean_bf = consts.tile([P, 1], BF16)    # 1/C  -> ones-matmul = mean
        nc.vector.memset(ones_mean_bf, 1.0 / C)
        ones_row = consts.tile([1, P], F32)         # 1.0  -> partition broadcast matmul
        nc.vector.memset(ones_row, 1.0)

        bqk_sb = consts.tile([P, 16], F32)
        nc.sync.dma_start(out=bqk_sb, in_=io["bqk"][:])
        bv_sb = consts.tile([P, C], F32)
        nc.sync.dma_start(out=bv_sb, in_=io["bv"][:])
        bcp_sb = consts.tile([P, KO], F32)
        nc.sync.dma_start(out=bcp_sb, in_=io["bcp"][:])
        bfc_sb = consts.tile([P, 32], F32)
        nc.sync.dma_start(out=bfc_sb, in_=io["bfc"][:])
        bpj_sb = consts.tile([P, KO], F32)
        nc.sync.dma_start(out=bpj_sb, in_=io["bpj"][:])

        mask_sb = p_act.tile([P, 2, T2], BF16, tag="mask")   # kc-pair packed tril
        nc.sync.dma_start(out=mask_sb, in_=io["mask"][:])
        ebias_sb = consts.tile([P, 1], F32)
        nc.sync.dma_start(out=ebias_sb, in_=io["ebias"][:])

        # ---- load x^T: bf16 full (LN/QKV path) + f32 own half (residual) ----
        xt_own = p_res.tile([P, KO, TOWN], F32, tag="xown")
        x_bf = p_big.tile([P, KO, T], BF16, tag="big")
        for ko in range(KO):
            nc.gpsimd.dma_start(out=x_bf[:, ko, :], in_=io["x_bf"][:, ko, :])
            nc.sync.dma_start(out=xt_own[:, ko, :], in_=io["xt_own"][:, ko, :])

        # ---- LayerNorm (stats across partitions via bf16 ones-matmuls) ----
        xln = p_big.tile([P, KO, T], BF16, tag="big")

        def emit_ln(srcs, dst, src_is_bf, stats_ps=None):
            """srcs: list of (tile, col0); normalizes [P,KO,TOWN] col-slices."""
            for s, (st, sc0) in enumerate(srcs):
                if stats_ps is not None:
                    st_ps = stats_ps
                else:
                    st_ps = ps_mm.tile([P, T2], F32, tag="mm")
                mu_ps = st_ps[0:1, 0:TOWN]
                sq_ps = st_ps[0:1, TOWN:T2]
                for ko in range(0 if stats_ps is not None else KO):
                    if src_is_bf:
                        xb = st[:, ko, sc0:sc0 + TOWN]
                    else:
                        xb = p_scr.tile([P, TOWN], BF16, tag="scr")
                        nc.scalar.copy(xb, st[:, ko, sc0:sc0 + TOWN])
                    sq = p_scr.tile([P, TOWN], BF16, tag="scr")
                    nc.vector.tensor_mul(sq, xb, xb)
                    nc.tensor.matmul(mu_ps, ones_mean_bf, xb,
                                     start=(ko == 0), stop=(ko == KO - 1))
                    nc.tensor.matmul(sq_ps, ones_mean_bf, sq,
                                     start=(ko == 0), stop=(ko == KO - 1))
                mu = p_row.tile([1, TOWN], F32, tag="row")
                nc.scalar.copy(mu, mu_ps)
                msq = p_row.tile([1, TOWN], F32, tag="row")
                nc.scalar.copy(msq, sq_ps)

                # rstd = 1 / (sqrt(msq - mu^2) + 1e-5)
                t = p_row.tile([1, TOWN], F32, tag="row")
                nc.vector.tensor_mul(t, mu, mu)
                nc.vector.tensor_sub(t, msq, t)
                nc.scalar.activation(t, t, Act.Sqrt)
                nc.vector.tensor_scalar_add(t, t, 1e-5)
                rstd = p_row.tile([1, TOWN], F32, tag="row")
                nc.vector.reciprocal_approx_fast(rstd, t)

                bc_ps = ps_mm.tile([P, T2], F32, tag="mm")
                mu_bc = bc_ps[:, 0:TOWN]
                rs_bc = bc_ps[:, TOWN:T2]
                nc.tensor.matmul(mu_bc, ones_row, mu, start=True, stop=True)
                nc.tensor.matmul(rs_bc, ones_row, rstd, start=True, stop=True)

                for ko in range(KO):
                    tt = p_scr.tile([P, TOWN], F32, tag="scr")
                    nc.vector.tensor_sub(tt, st[:, ko, sc0:sc0 + TOWN], mu_bc)
                    nc.vector.tensor_mul(dst[:, ko, sc0:sc0 + TOWN], tt, rs_bc)

        emit_ln([(x_bf, 0), (x_bf, TOWN)], xln, True)

        # ---- QKV projections (q^T, k^T transposed; v natural) ----
        qT = p_act.tile([P, KO, TOWN], BF16, tag="qT")
        kT = p_act.tile([P, KO, T], BF16, tag="kT")
        # q: pairs of output-channel chunks share one 2-bank psum tile
        for mop in range(4):
            ps = ps_mm.tile([P, T2], F32, tag="mm")
            for half in range(2):
                mo = 2 * mop + half
                wt = p_wqk.tile([P, KO, P], BF16, tag="wqk")
                (nc.sync if mo % 2 == 0 else nc.gpsimd).dma_start(
                    out=wt, in_=io["wqk"][mo])
                for ko in range(KO):
                    nc.tensor.matmul(ps[:, half * TOWN:(half + 1) * TOWN],
                                     wt[:, ko, :], xln[:, ko, 0:TOWN],
                                     start=(ko == 0), stop=(ko == KO - 1))
            for half in range(2):
                mo = 2 * mop + half
                nc.scalar.activation(qT[:, mo, :],
                                     ps[:, half * TOWN:(half + 1) * TOWN],
                                     Act.Identity, bias=bqk_sb[:, mo:mo + 1])
        # k: one chunk's own+oth halves share a tile; single batched evict
        for mo in range(8, 16):
            wt = p_wqk.tile([P, KO, P], BF16, tag="wqk")
            (nc.sync if mo % 2 == 0 else nc.gpsimd).dma_start(
                out=wt, in_=io["wqk"][mo])
            ps = ps_mm.tile([P, T2], F32, tag="mm")
            for half in range(2):
                for ko in range(KO):
                    nc.tensor.matmul(ps[:, half * TOWN:(half + 1) * TOWN],
                                     wt[:, ko, :],
                                     xln[:, ko, half * TOWN:(half + 1) * TOWN],
                                     start=(ko == 0), stop=(ko == KO - 1))
            nc.scalar.activation(kT[:, mo - 8, :], ps, Act.Identity,
                                 bias=bqk_sb[:, mo:mo + 1])

        v_ext = p_act.tile([P, KO, 16 * 65], BF16, tag="v")
        nc.vector.memset(v_ext, 1.0)
        for nh in range(2):
            wvt = []
            for ko in range(KO):
                w = p_wv.tile([P, TOWN], BF16, tag="wv")
                (nc.sync if ko % 2 == 0 else nc.gpsimd).dma_start(
                    out=w, in_=io["wv"][ko, nh])
                wvt.append(w)
            for tkbp in range(4):
                ps = ps_mm.tile([P, T2], F32, tag="mm")
                for half in range(2):
                    tkb = 2 * tkbp + half
                    for ko in range(KO):
                        nc.tensor.matmul(ps[:, half * TOWN:(half + 1) * TOWN],
                                         xln[:, ko, tkb * P:(tkb + 1) * P],
                                         wvt[ko],
                                         start=(ko == 0), stop=(ko == KO - 1))
                for half in range(2):
                    tkb = 2 * tkbp + half
                    vout = v_ext[:, tkb].rearrange("p (h d) -> p h d", d=65)
                    nc.vector.tensor_add(
                        vout[:, nh * 8:(nh + 1) * 8, 0:64],
                        ps[:, half * TOWN:(half + 1) * TOWN].rearrange(
                            "p (h d) -> p h d", d=64),
                        bv_sb[:, nh * TOWN:(nh + 1) * TOWN].rearrange(
                            "p (h d) -> p h d", d=64))

        # ---- attention ----
        yT = p_act.tile([P, KO, TOWN], BF16, tag="yT")
        all_pts = {}

        def emit_scores(hp):
            for i in range(2):              # head 2hp+i at partitions 64i:64i+64
                pb = 64 * i
                for kcp in range(4):        # kc pair (2kcp, 2kcp+1)
                    ps = ps_mm.tile([P, T2], F32, tag="mm")
                    for half in range(2):
                        kc = 2 * kcp + half
                        nc.tensor.matmul(ps[:, half * TOWN:(half + 1) * TOWN],
                                         kT[pb:pb + 64, hp, kc * P:(kc + 1) * P],
                                         qT[pb:pb + 64, hp, :],
                                         start=True, stop=True)
                    pt = p_pt.tile([P, T2], BF16, tag="pt")
                    if kcp < 2:
                        nc.scalar.activation(pt, ps, Act.Exp)
                        nc.vector.tensor_mul(pt, pt, mask_sb[:, kcp, :])
                    else:
                        nc.scalar.activation(pt, ps, Act.Exp,
                                             bias=ebias_sb[:, 0:1])
                    all_pts[(hp, i, kcp)] = pt

        def emit_av(hp):
            psy_a = ps_av.tile([P, TOWN], F32, tag="av")
            psy_b = ps_av.tile([P, TOWN], F32, tag="av")
            psy = [psy_a, psy_b]
            for i in range(2):
                hd = 2 * hp + i
                for kc in range(KO):
                    pt = all_pts[(hp, i, kc // 2)]
                    nc.tensor.matmul(psy[i][0:65, :],
                                     v_ext[:, kc, hd * 65:(hd + 1) * 65],
                                     pt[:, (kc % 2) * TOWN:(kc % 2 + 1) * TOWN],
                                     start=(kc == 0), stop=(kc == KO - 1))
            for i in range(2):
                pb = 64 * i
                z = p_row.tile([1, TOWN], F32, tag="zrow")
                nc.vector.tensor_copy(z, psy[i][64:65, :])
                rz = p_row.tile([1, TOWN], F32, tag="zrow")
                nc.vector.reciprocal_approx_fast(rz, z)
                rzbc = p_scr.tile([P, TOWN], F32, tag="scr")
                nc.gpsimd.partition_broadcast(rzbc, rz, channels=P)
                nc.vector.tensor_mul(yT[pb:pb + 64, hp, :], psy[i][0:64, :],
                                     rzbc[0:64, :])

        emit_scores(0)
        for hp in range(1, 8):
            emit_scores(hp)
            emit_av(hp - 1)
        emit_av(7)

        # ---- c_proj + residual (x2 written in place over xt_own) ----
        for mop in range(4):
            ps = ps_mm.tile([P, T2], F32, tag="mm")
            for half in range(2):
                mo = 2 * mop + half
                wt = p_wcp.tile([P, KO, P], BF16, tag="wcp")
                nc.sync.dma_start(out=wt, in_=io["wcp"][mo])
                for ko in range(KO):
                    nc.tensor.matmul(ps[:, half * TOWN:(half + 1) * TOWN],
                                     wt[:, ko, :], yT[:, ko, :],
                                     start=(ko == 0), stop=(ko == KO - 1))
            for half in range(2):
                mo = 2 * mop + half
                nc.vector.scalar_tensor_tensor(
                    xt_own[:, mo, :], ps[:, half * TOWN:(half + 1) * TOWN],
                    bcp_sb[:, mo:mo + 1], xt_own[:, mo, :],
                    op0=Alu.add, op1=Alu.add)

        # ---- LN2 + MLP ----
        x2ln = p_act.tile([P, KO, TOWN], BF16, tag="x2ln")
        emit_ln([(xt_own, 0)], x2ln, False)

        h0 = p_big.tile([P, 16, TOWN], BF16, tag="big")
        h1 = p_big.tile([P, 16, TOWN], BF16, tag="big")
        hh = [h0, h1]
        for mop in range(16):
            ps = ps_mm.tile([P, T2], F32, tag="mm")
            for half in range(2):
                mo = 2 * mop + half
                wt = p_wfc.tile([P, KO, P], BF16, tag="wfc")
                (nc.sync if mo % 2 == 0 else nc.gpsimd).dma_start(
                    out=wt, in_=io["wfc"][mo])
                for ko in range(KO):
                    nc.tensor.matmul(ps[:, half * TOWN:(half + 1) * TOWN],
                                     wt[:, ko, :], x2ln[:, ko, :],
                                     start=(ko == 0), stop=(ko == KO - 1))
            for half in range(2):
                mo = 2 * mop + half
                nc.scalar.activation(hh[mo // 16][:, mo % 16, :],
                                     ps[:, half * TOWN:(half + 1) * TOWN],
                                     Act.Gelu, bias=bfc_sb[:, mo:mo + 1])

        for mop in range(4):
            ps = ps_mm.tile([P, T2], F32, tag="mm")
            for half in range(2):
                mo = 2 * mop + half
                wts = []
                for whalf in range(2):
                    wt = p_wpj.tile([P, 16, P], BF16, tag="wpj")
                    (nc.sync if whalf == 0 else nc.gpsimd).dma_start(
                        out=wt, in_=io["wpj"][mo][:, whalf * 16:(whalf + 1) * 16, :])
                    wts.append(wt)
                for ko in range(32):
                    nc.tensor.matmul(ps[:, half * TOWN:(half + 1) * TOWN],
                                     wts[ko // 16][:, ko % 16, :],
                                     hh[ko // 16][:, ko % 16, :],
                                     start=(ko == 0), stop=(ko == 31))
            for half in range(2):
                mo = 2 * mop + half
                ot = p_out.tile([P, TOWN], F32, tag="outst")
                nc.vector.scalar_tensor_tensor(ot, ps[:, half * TOWN:(half + 1) * TOWN],
                                               bpj_sb[:, mo:mo + 1],
                                               xt_own[:, mo, :],
                                               op0=Alu.add, op1=Alu.add)
                nc.sync.dma_start(out=io["out"][:, mo, :], in_=ot)


def _build_nc():
    nc = bacc.Bacc("TRN2", target_bir_lowering=False, debug=False)
    io = {}
    dt = nc.dram_tensor
    io["xt_own"] = dt("xt_own", [P, KO, TOWN], F32, kind="ExternalInput")
    io["x_bf"] = dt("x_bf", [P, KO, T], BF16, kind="ExternalInput")
    io["wqk"] = dt("wqk", [16, P, KO, P], BF16, kind="ExternalInput")
    io["wv"] = dt("wv", [KO, 2, P, TOWN], BF16, kind="ExternalInput")
    io["wcp"] = dt("wcp", [KO, P, KO, P], BF16, kind="ExternalInput")
    io["wfc"] = dt("wfc", [32, P, KO, P], BF16, kind="ExternalInput")
    io["wpj"] = dt("wpj", [KO, P, 32, P], BF16, kind="ExternalInput")
    io["bqk"] = dt("bqk", [P, 16], F32, kind="ExternalInput")
    io["bv"] = dt("bv", [P, C], F32, kind="ExternalInput")
    io["bcp"] = dt("bcp", [P, KO], F32, kind="ExternalInput")
    io["bfc"] = dt("bfc", [P, 32], F32, kind="ExternalInput")
    io["bpj"] = dt("bpj", [P, KO], F32, kind="ExternalInput")
    io["mask"] = dt("mask", [P, 2, T], BF16, kind="ExternalInput")
    io["ebias"] = dt("ebias", [P, 1], F32, kind="ExternalInput")
    io["out"] = dt("out", [P, KO, TOWN], F32, kind="ExternalOutput")
    with tile.TileContext(nc) as tc:
        _emit(nc, tc, io)
    nc.compile()
    return nc


def _prep_maps(inputs):
    f32 = np.float32
    g = {k: np.asarray(v, f32) for k, v in inputs.items()}

    # fold LN gains/biases into the following projections
    Wa = g["c_attn_w"] * g["ln1_w"][:, None]
    ba = g["c_attn_b"] + g["ln1_b"] @ g["c_attn_w"]
    Wq, Wk, Wv = Wa[:, :C] * 0.125, Wa[:, C:2 * C], Wa[:, 2 * C:]
    bq, bk, bv = ba[:C] * 0.125, ba[C:2 * C], ba[2 * C:]
    Wfc = g["fc_w"] * g["ln2_w"][:, None]
    bfc = g["fc_b"] + g["ln2_b"] @ g["fc_w"]

    def lhsT_arrange(w, n_mo):  # [C_in, N] -> [n_mo, P(ki), KO_in, P(mi)] bf16
        ko_in = w.shape[0] // P
        return np.ascontiguousarray(
            w.reshape(ko_in, P, n_mo, P).transpose(2, 1, 0, 3)).astype(np_bf16)

    shared = {
        "wqk": lhsT_arrange(np.concatenate([Wq, Wk], axis=1), 16),
        "wv": np.ascontiguousarray(
            Wv.reshape(KO, P, 2, TOWN).transpose(0, 2, 1, 3)).astype(np_bf16),
        "wcp": lhsT_arrange(g["c_proj_w"], KO),
        "wfc": lhsT_arrange(Wfc, 32),
        "wpj": lhsT_arrange(g["proj_w"], KO),
        "bqk": np.ascontiguousarray(
            np.concatenate([bq, bk]).reshape(16, P).T).astype(f32),
        "bv": np.ascontiguousarray(np.broadcast_to(bv, (P, C))).astype(f32),
        "bcp": np.ascontiguousarray(g["c_proj_b"].reshape(KO, P).T).astype(f32),
        "bfc": np.ascontiguousarray(bfc.reshape(32, P).T).astype(f32),
        "bpj": np.ascontiguousarray(g["proj_b"].reshape(KO, P).T).astype(f32),
    }

    maps = []
    gq_base = np.arange(TOWN)
    gk_base = np.arange(T)
    for c in range(8):
        b, h = divmod(c, 2)
        xr = np.roll(g["x"][b], -h * TOWN, axis=0)          # own tokens first
        arr = np.ascontiguousarray(
            xr.T.reshape(KO, P, T).transpose(1, 0, 2)).astype(f32)  # [P, KO, T]
        m = (gk_base[:TOWN, None] <= gq_base[None, :]).astype(f32)  # tril [TOWN, TOWN]
        # [P(ki), kcp, half*TOWN+q] with key = (2*kcp+half)*P + ki
        mask = np.ascontiguousarray(
            m.reshape(2, 2, P, TOWN).transpose(2, 0, 1, 3).reshape(P, 2, T)
        ).astype(np_bf16)
        ebias = np.full((P, 1), -50.0 if h == 0 else 0.0, f32)
        maps.append(dict(shared,
                         xt_own=np.ascontiguousarray(arr[:, :, :TOWN]),
                         x_bf=arr.astype(np_bf16),
                         mask=mask, ebias=ebias))
    return maps


def kernel(**inputs):
    global LAST_RESULTS, _NC_CACHE
    if _NC_CACHE is None:
        _NC_CACHE = _build_nc()
    nc = _NC_CACHE
    maps = _prep_maps(inputs)
    res = run_bass_kernel_spmd(nc, maps, core_ids=list(range(8)),
                               trace=TRACE, **TRACE_KW)
    LAST_RESULTS = res
    out = np.zeros((B, T, C), np.float32)
    for c in range(8):
        b, h = divmod(c, 2)
        ot = res.results[c]["out"]                # [P, KO, TOWN]
        out[b, h * TOWN:(h + 1) * TOWN, :] = \
            ot.transpose(1, 0, 2).reshape(C, TOWN).T
    return out
